# revision 45
# baseline (speedup 1.0000x reference)
"""Causal multi-head attention layer for Trainium2 (Bass/Tile), 8 NeuronCores.

Problem: x[B=2,S=2048,D=1024], H=16 heads, Dh=64.
Sharding: data-parallel over batch (2) x tensor-parallel over head groups (4):
each of the 8 cores handles one batch element and 4 heads, producing a partial
output [S, D]; the host sums the 4 head-group partials per batch (the
"all-reduce after the W_O contraction" done host-side since we return full
output anyway) and adds biases that commute out (b_O and sum_h b_V[h] @ W_O[h],
exact because softmax rows sum to 1).

Device kernel (per core), all operands resident in SBUF:
  - x^T is fed pre-transposed from host: [128, KT=8, S] (D on partitions).
  - Q^T, K^T computed head-PAIR-packed: [128, NPAIR, S] (partitions 0:64 =
    head 2*pr dims, 64:128 = head 2*pr+1). W as stationary [128,128], x^T
    moving N=512.
  - V computed in [k, e] layout (x^T stationary, W_V moving N=256, all 4
    heads at once) and stored with an appended [1, 0] column pair: V'=[V|1|0].
  - Scores computed TRANSPOSED: S^T[k, q] = (K^T tile).T @ Q^T chunk, so
    softmax's sum lands on the matmul contraction instead of needing row
    reductions: Z'[e|1|0, q] = V'.T @ exp(S^T) accumulated over k-tiles gives
    both the unnormalized attention output (rows 0:64) and the softmax
    denominator l (row 64) in one accumulation. No max-subtraction is needed:
    scores are O(1) here, exp is safe in fp32.
  - Both heads of a pair write one 2-bank PSUM tile (disjoint PE row groups,
    so their K=64 matmuls run concurrently) and share a single 1024-wide
    ACTIVATE(Exp) to amortize the ~480ns ACT fixed cost.
  - Causal masking is multiplicative on exp(S^T), diagonal 128-stripes only
    (on GpSimd/Pool, which has slack); one shared [128,128] mask serves every
    diagonal stripe since the stripe-local condition is q_local >= k for all
    of them. Fully-masked column ranges of diagonal chunks are skipped in the
    scores/exp/PV instructions.
  - The ones block of V' is replicated 64x, so l lands pre-broadcast on
    PV-accumulator partitions 64:128 and normalization is a wide DVE
    reciprocal_approx_fast + multiply — no cross-partition traffic. (The
    approx reciprocal must read the multi-matmul PSUM accumulation via an
    SBUF staging copy; reading PSUM directly returns garbage on HW.)
  - The whole projection phase is PIPELINED INTO the flash loop: only Q/K for
    (chunk 0, pair 0) + V tile 0 run up front (paced by the first 2MB of DMA);
    every other projection group (Q/K per (chunk, pair, QK), V per k-tile) and
    every output-projection step is a closure in a due-date-ordered fill
    queue, popped one per flash j-iteration after the PV matmuls. Due dates
    (K before its chunk/pair section, V tile j before slot j of the first
    pair) are enforced by draining the queue head at section/slot boundaries.
    PE program order then guarantees the data is ready.
  - The output projection (single K=128 matmuls per head pair — the pair-sum
    rides the contraction) is evicted PSUM->SBUF as f16 on the VECTOR engine
    (ACT does fp32 copies at 2 cyc/elem; DVE is ~2.4x faster and ACT is the
    flash pacer), and the DRAM output tensor is f16 (halves output DMA; the
    host accumulates partials in fp32).
  - Input DMAs are interleaved ktile-by-ktile in first-use order, with the
    chunk-0 x^T slice split out so the first Q/K groups start after ~2MB.
"""

import os
from collections import deque

import numpy as np

# 'f16'   = float16 operands: 2-byte moving operand streams at 1 PE
#           cycle/row (4-byte fp32/fp32r cost 2), 11-bit mantissa
# 'fp32r' = fp32 bits, single-pass reduced-precision PE mode (2 cyc/row)
# 'bf16'  = bf16 storage/matmuls (1 cyc/row, 8-bit mantissa)
# 'fp32'  = exact fp32 matmuls (two-pass, 4 cyc/row)
MM_MODE = os.environ.get("ATTN_MM_MODE", "f16")

P = 128
SC = 512  # q-chunk width (one PSUM bank of fp32)

_BUILD_CACHE = {}


def _np_sb(mm_mode):
    if mm_mode == "bf16":
        import ml_dtypes

        return np.dtype(ml_dtypes.bfloat16)
    if mm_mode == "f16":
        return np.dtype(np.float16)
    return np.dtype(np.float32)


def build_nc(S, Dm, NH, Dh, mm_mode, stage=99):
    """Build (and cache) the per-core Bass module. NH = heads per core."""
    key = (S, Dm, NH, Dh, mm_mode, stage)
    if key in _BUILD_CACHE:
        return _BUILD_CACHE[key]

    import concourse.bacc as bacc
    import concourse.mybir as mybir
    import concourse.tile as tile

    f32 = mybir.dt.float32
    dt_w = {
        "bf16": mybir.dt.bfloat16,
        "f16": mybir.dt.float16,
        "fp32": mybir.dt.float32,
        "fp32r": mybir.dt.float32r,
    }[mm_mode]
    # dtype for non-matmul elementwise tiles (masks) and the DRAM output
    dt_m = {
        "bf16": mybir.dt.bfloat16,
        "f16": mybir.dt.float16,
    }.get(mm_mode, mybir.dt.float32)

    KT = Dm // P       # k-tiles over the model dim (contraction of projections)
    NPAIR = NH // 2    # head pairs
    QC = S // SC       # q chunks
    NKT = S // P       # k-position tiles
    DH2 = Dm // SC     # output free-dim chunks
    TPC = SC // P      # q-tiles per chunk
    assert Dh == 64 and NH % 2 == 0 and S % SC == 0 and Dm % SC == 0

    nc = bacc.Bacc(
        "TRN2",
        debug=False,
        enable_asserts=False,
        target_bir_lowering=False,
        num_devices=1,
    )

    xT_d = nc.dram_tensor("xT", [P, KT, S], dt_w, kind="ExternalInput")
    wqk_d = nc.dram_tensor("wqk", [P, KT, 2, NPAIR, P], dt_w, kind="ExternalInput")
    wv_d = nc.dram_tensor("wv", [P, KT, NH * Dh], dt_w, kind="ExternalInput")
    wo_d = nc.dram_tensor("wo", [P, NPAIR, Dm], dt_w, kind="ExternalInput")
    bqk_d = nc.dram_tensor("bqk", [P, 2, NPAIR], f32, kind="ExternalInput")
    out_d = nc.dram_tensor("out", [S, Dm], dt_m, kind="ExternalOutput")

    def mm(ap):
        return ap

    Exp = mybir.ActivationFunctionType.Exp
    inv_sqrt_dh = 1.0 / float(np.sqrt(Dh))

    with tile.TileContext(nc) as tc:
        with tc.tile_pool(name="const", bufs=1) as cpool:
            # ---------- persistent tiles ----------
            wqk = cpool.tile([P, KT, 2, NPAIR, P], dt_w)
            wv = cpool.tile([P, KT, NH * Dh], dt_w)
            wo = cpool.tile([P, NPAIR, Dm], dt_w)
            bqk = cpool.tile([P, 2, NPAIR], f32)
            xT = cpool.tile([P, KT, S], dt_w)

            QTt = cpool.tile([P, NPAIR, S], dt_w)
            KTt = cpool.tile([P, NPAIR, S], dt_w)
            # V' = [V | 1...1]: the ones block is REPLICATED 64x so the PV
            # matmul broadcasts the softmax denominator l across output
            # partitions 64:128.
            Vt = cpool.tile([P, NKT, NH, 2 * Dh], dt_w)

            # ONE shared causal mask stripe: within any diagonal 128-stripe v
            # (global cols [v*128,(v+1)*128)), keep iff q_local >= k, i.e.
            # col - partition >= 0 — identical for every v.
            mask1 = cpool.tile([P, P], dt_m)
            nc.gpsimd.memset(mask1[:], 1.0)
            nc.gpsimd.affine_select(
                out=mask1[:],
                in_=mask1[:],
                compare_op=mybir.AluOpType.is_ge,
                fill=0.0,
                base=0,
                pattern=[[1, P]],
                channel_multiplier=-1,
            )

            # ---------- DMA, in first-use order ----------
            # DMA descriptors issue serially at ~0.65us each on the Sync
            # engine REGARDLESS of size (the 16 data rings swallow the bytes),
            # so batch coarsely: phase A = pair-0 Q/K weights + the chunk-0
            # x slice in ONE descriptor each, then the rest sliced to match
            # the order the interleaved fill work consumes it.
            nc.sync.dma_start(wqk[:, :, :, 0, :], wqk_d[:, :, :, 0, :])
            # chunk-0 x slice split by k-tile halves so the first Q/K matmuls
            # start while the second half is still in flight; wv lands between
            # them so the V(0) group can interleave into the Q/K tail
            nc.sync.dma_start(
                xT[:, 0 : KT // 2, 0:SC], xT_d[:, 0 : KT // 2, 0:SC]
            )
            nc.sync.dma_start(
                xT[:, KT // 2 : KT, 0:SC], xT_d[:, KT // 2 : KT, 0:SC]
            )
            nc.sync.dma_start(bqk[:], bqk_d[:])
            nc.sync.dma_start(wv[:], wv_d[:])
            for pr in range(1, NPAIR):
                nc.sync.dma_start(wqk[:, :, :, pr, :], wqk_d[:, :, :, pr, :])
            for qcx in range(1, QC):
                qsx = slice(qcx * SC, (qcx + 1) * SC)
                nc.sync.dma_start(xT[:, :, qsx], xT_d[:, :, qsx])
                if qcx == 2:
                    nc.sync.dma_start(wo[:], wo_d[:])

            # ---------- warmup + minimal phase A ----------
            with (
                tc.tile_pool(name="wu", bufs=1) as wupool,
                tc.tile_pool(name="psA", bufs=5, space="PSUM") as psA,
            ):
                # HAM warm-up: dummy matmuls during the initial DMA wait so
                # the PE clock-gate is ramping while data streams in
                wst = wupool.tile([P, SC], f32)
                nc.vector.memset(wst[:], 1.0)
                # preload the Exp table on the Scalar engine now (it's idle);
                # otherwise the first flash exp pays the ~2.7us table load
                # on the critical path
                tpre = wupool.tile([1, 2], f32)
                nc.scalar.activation(tpre[:], wst[0:1, 0:2], Exp)
                wrm = wupool.tile([P, SC], dt_w)
                nc.vector.tensor_copy(wrm[:], wst[:])
                nwu = 4
                pwu = psA.tile([P, SC], f32, tag="mm")
                for i in range(nwu):
                    nc.tensor.matmul(
                        pwu[:], mm(wrm[:, 0:P]), mm(wrm[:]),
                        start=(i == 0), stop=(i == nwu - 1),
                    )

                # phase A: only Q/K for (chunk 0, pair 0) and V tile 0 —
                # everything else is fill work inside the flash loop.
                # Warm-up matmuls stay interleaved between the DMA-paced
                # k-tile groups: they execute while the next k-tile is still
                # landing, keeping the clock-gate ramping so the real
                # matmuls run at full speed.
                psqk = {
                    pj: psA.tile([P, SC], f32, tag="mm", name=f"psA_{pj}")
                    for pj in range(2)
                }
                pwu2 = psA.tile([P, SC], f32, tag="mm", name="pwu2")
                nwu2 = 0
                for kt in range(KT):
                    for pj in range(2):
                        nc.tensor.matmul(
                            psqk[pj][:],
                            mm(wqk[:, kt, pj, 0, :]), mm(xT[:, kt, 0:SC]),
                            start=(kt == 0), stop=(kt == KT - 1),
                        )
                    if kt < KT - 1:
                        nc.tensor.matmul(
                            pwu2[:], mm(wrm[:, 0:P]), mm(wrm[:]),
                            start=(nwu2 == 0), stop=(kt == KT - 2),
                        )
                        nwu2 += 1
                psV = psA.tile([P, NH * Dh], f32, tag="mm")
                for kt in range(KT):
                    nc.tensor.matmul(
                        psV[:],
                        mm(xT[:, kt, 0:P]), mm(wv[:, kt, :]),
                        start=(kt == 0), stop=(kt == KT - 1),
                    )
                # both evictions gate the first scores pair: Q on Vector, K
                # on Scalar (idle here) so they run in parallel
                nc.vector.tensor_scalar_add(
                    QTt[:, 0, 0:SC], psqk[0][:], bqk[:, 0, 0:1]
                )
                nc.scalar.activation(
                    KTt[:, 0, 0:SC], psqk[1][:],
                    mybir.ActivationFunctionType.Identity,
                    bias=bqk[:, 1, 0:1],
                )

                nc.vector.tensor_copy(
                    Vt[:, 0, :, 0:Dh],
                    psV[:].rearrange("p (h e) -> p h e", e=Dh),
                )

                # memset can't write float32r: stage the V' ones in f32, copy
                # over with a free-dim broadcast (needed first by the PV
                # matmuls of the first chunk)
                cstage = wupool.tile([P, 1, 1, Dh], f32)
                nc.vector.memset(cstage[:], 1.0)
                nc.vector.tensor_copy(
                    Vt[:, :, :, Dh : 2 * Dh],
                    cstage[:].to_broadcast((P, NKT, NH, Dh)),
                )

            # ---------- flash + interleaved projections/output ----------
            with tc.tile_pool(name="zt", bufs=1) as ztpool:
                ZTt = ztpool.tile([P, NPAIR, S], dt_w)
                self_flash(
                    nc, tc, stage, mm, Exp, inv_sqrt_dh, mybir,
                    QTt, KTt, Vt, ZTt, wo, out_d, mask1, xT, wv, wqk, bqk,
                    S, Dm, Dh, NPAIR, QC, SC, P, DH2, KT, NKT, TPC,
                    dt_w, dt_m, f32,
                )

    nc.compile()
    _BUILD_CACHE[key] = nc
    return nc


def self_flash(
    nc, tc, stage, mm, Exp, inv_sqrt_dh, mybir,
    QTt, KTt, Vt, ZTt, wo, out_d, mask1, xT, wv, wqk, bqk,
    S, Dm, Dh, NPAIR, QC, SC, P, DH2, KT, NKT, TPC,
    dt_w, dt_m, f32,
):
    NH = Vt.shape[2]
    with (
        tc.tile_pool(name="e", bufs=4) as epool,
        tc.tile_pool(name="r", bufs=4) as rpool,
        tc.tile_pool(name="o", bufs=4) as opool,
        tc.tile_pool(name="pss", bufs=2, space="PSUM") as ps_s,
        tc.tile_pool(name="psz", bufs=4, space="PSUM") as psz,
    ):
        # ----- fill-work closures -----
        def qk_step(qc, pr, pj):
            """One deferred Q/K projection group (8 matmuls + DVE evict)."""
            def emit():
                ps = psz.tile([P, SC], f32, tag="z", name=f"psqk_{qc}_{pr}_{pj}")
                qs = slice(qc * SC, (qc + 1) * SC)
                for kt in range(KT):
                    nc.tensor.matmul(
                        ps[:], mm(wqk[:, kt, pj, pr, :]), mm(xT[:, kt, qs]),
                        start=(kt == 0), stop=(kt == KT - 1),
                    )
                dst = QTt if pj == 0 else KTt
                nc.vector.tensor_scalar_add(
                    dst[:, pr, qs], ps[:], bqk[:, pj, pr : pr + 1]
                )
            return emit

        def v_step(qt):
            """One deferred V-projection group (8 matmuls + DVE evict)."""
            def emit():
                psV = psz.tile([P, NH * Dh], f32, tag="z", name=f"psv_{qt}")
                for kt in range(KT):
                    nc.tensor.matmul(
                        psV[:],
                        mm(xT[:, kt, qt * P : (qt + 1) * P]), mm(wv[:, kt, :]),
                        start=(kt == 0), stop=(kt == KT - 1),
                    )
                nc.vector.tensor_copy(
                    Vt[:, qt, :, 0:Dh],
                    psV[:].rearrange("p (h e) -> p h e", e=Dh),
                )
            return emit

        ot_stage = {}

        def op_step(t, dh2, evict_act=False):
            """One output-projection step: out[q-tile t, d-chunk dh2] =
            sum_pr ZT-pair @ W_O-pair, evicted f16 into a per-tile staging
            buffer; the last d-chunk DMAs the whole tile (one descriptor per
            q-tile — descriptors cost ~0.65us on Sync regardless of size)."""
            def emit():
                zs = slice(t * P, (t + 1) * P)
                if dh2 == 0:
                    ot_stage[t] = opool.tile(
                        [P, Dm], dt_m, tag="o", name=f"ot_{t}"
                    )
                ot = ot_stage[t]
                po = psz.tile([P, SC], f32, tag="z", name=f"po_{t}_{dh2}")
                ds = slice(dh2 * SC, (dh2 + 1) * SC)
                for pr in range(NPAIR):
                    nc.tensor.matmul(
                        po[:], mm(ZTt[:, pr, zs]), mm(wo[:, pr, ds]),
                        start=(pr == 0), stop=(pr == NPAIR - 1),
                    )
                if evict_act:
                    nc.scalar.activation(
                        ot[:, ds], po[:], mybir.ActivationFunctionType.Copy
                    )
                else:
                    nc.vector.tensor_copy(ot[:, ds], po[:])
                if dh2 == DH2 - 1:
                    nc.sync.dma_start(out_d[t * P : (t + 1) * P, :], ot[:])
                    del ot_stage[t]
            return emit

        # fill queue entries: (due, earliest, emit). due: ('k', (qc, pr))
        # must complete before section (qc, pr); ('v', qt) before slot j=qt
        # of the FIRST pair of chunk qt//TPC. earliest: the section from
        # which the item may be popped voluntarily — keeps work for chunk c
        # out of the (already PE-heavy, exp-light) chunks before c-1, so the
        # long exp streams of late chunks get the fill they can hide.
        fill_q = deque()
        fill_q.append(((("k", (0, 1)), (0, 0)), qk_step(0, 1, 0)))
        fill_q.append(((("k", (0, 1)), (0, 0)), qk_step(0, 1, 1)))
        for qt in range(1, TPC):
            fill_q.append(((("v", qt), (0, 0)), v_step(qt)))
        for qc in range(1, QC):
            erl = (max(qc - 2, 0), 0)
            for pr in range(NPAIR):
                for pj in range(2):
                    fill_q.append(
                        ((("k", (qc, pr)), erl), qk_step(qc, pr, pj))
                    )
            for qt in range(qc * TPC, (qc + 1) * TPC):
                fill_q.append(((("v", qt), erl), v_step(qt)))
        op_queue = deque()

        def pop_fill(sec, allow_op=True):
            for i, item in enumerate(fill_q):
                if item[0][1] <= sec:
                    del fill_q[i]
                    item[1]()
                    return True
            if allow_op and op_queue:
                op_step(*op_queue.popleft())()
                return True
            return False

        def drain_due(qc, pr, j):
            """Force-emit overdue fill items (whole-queue scan, so a not-due
            head can't block a due item behind it). K for section (qc,pr)
            must be emitted before its scores; V tile j before slot j of the
            first pair (PV(j) is emitted at slot j+1, after this drain)."""
            keep = []
            while fill_q:
                item = fill_q.popleft()
                kind, key = item[0][0]
                if (kind == "k" and key <= (qc, pr)) or (
                    kind == "v" and pr == 0 and key <= j
                ):
                    item[1]()
                else:
                    keep.append(item)
            fill_q.extend(keep)

        def normalize(pr, qc, zA, zB):
            """ZT[:, q] = Z'[0:64, q] * (1 / l[q]); l arrives pre-broadcast
            on partitions 64:128 of the PV accumulators. DVE-only.
            (reciprocal_approx_fast must not read multi-matmul PSUM
            accumulations directly — stage l through SBUF first.)"""
            qs = slice(qc * SC, (qc + 1) * SC)
            rb = rpool.tile([64, 2, SC], f32, tag="rb")
            ls = rpool.tile([64, 2, SC], f32, tag="ls")
            nc.vector.tensor_copy(ls[:, 0, :], zA[Dh : 2 * Dh, :])
            nc.vector.tensor_copy(ls[:, 1, :], zB[Dh : 2 * Dh, :])
            nc.vector.reciprocal_approx_fast(rb[:], ls[:])
            nc.vector.tensor_mul(ZTt[0:64, pr, qs], zA[0:Dh, :], rb[:, 0, :])
            nc.vector.tensor_mul(ZTt[64:128, pr, qs], zB[0:Dh, :], rb[:, 1, :])

        chunk_tail = None
        for qc in range(QC if stage >= 2 else 0):
            for pr in range(NPAIR):
                drain_due(qc, pr, -1)
                hA, hB = 2 * pr, 2 * pr + 1
                zA = psz.tile([P, SC], f32, tag="z")
                zB = psz.tile([P, SC], f32, tag="z")
                jmax = (qc + 1) * TPC
                pend = None

                def emit_pv(j, eAB, c0, jmax=jmax, zA=zA, zB=zB, hA=hA, hB=hB):
                    st, sp = j == 0, j == jmax - 1
                    cs = slice(c0, SC)
                    nc.tensor.matmul(
                        zA[:, cs], mm(Vt[:, j, hA, :]), mm(eAB[:, 0, cs]),
                        start=st, stop=sp,
                    )
                    nc.tensor.matmul(
                        zB[:, cs], mm(Vt[:, j, hB, :]), mm(eAB[:, 1, cs]),
                        start=st, stop=sp,
                    )

                for j in range(jmax):
                    if pr == 0:
                        drain_due(qc, pr, j)
                    v = j - (jmax - TPC)
                    # causal: columns below the diagonal tile are fully
                    # masked; skip them (fp32r moving dims must stay >= 256;
                    # 2-byte dtypes can slice all the way down)
                    if dt_w is mybir.dt.float32r:
                        c0 = min(v * P, 2 * P) if v > 0 else 0
                    else:
                        c0 = v * P if v > 0 else 0
                    cs = slice(c0, SC)
                    qf = slice(qc * SC + c0, (qc + 1) * SC)
                    sAB = ps_s.tile([P, 2, SC], f32, tag="s")
                    ks = slice(j * P, (j + 1) * P)
                    nc.tensor.matmul(
                        sAB[:, 0, cs],
                        mm(KTt[0:64, pr, ks]), mm(QTt[0:64, pr, qf]),
                        start=True, stop=True,
                    )
                    nc.tensor.matmul(
                        sAB[:, 1, cs],
                        mm(KTt[64:128, pr, ks]), mm(QTt[64:128, pr, qf]),
                        start=True, stop=True,
                    )
                    eAB = epool.tile([P, 2, SC], dt_w, tag="e")
                    nc.scalar.activation(
                        eAB[:, :, cs], sAB[:, :, cs], Exp, scale=inv_sqrt_dh
                    )
                    if v >= 0:  # chunk contains the causal diagonal
                        mv = slice(v * P, (v + 1) * P)
                        nc.gpsimd.tensor_mul(
                            eAB[:, 0, mv], eAB[:, 0, mv], mask1[:, :]
                        )
                        nc.gpsimd.tensor_mul(
                            eAB[:, 1, mv], eAB[:, 1, mv], mask1[:, :]
                        )
                    if stage >= 3:
                        if pend is not None:
                            emit_pv(*pend)
                        pend = (j, eAB, c0)
                        if j == 0 and chunk_tail is not None:
                            # cross-chunk pipeline: the previous chunk's last
                            # PV + normalize go here, AFTER this chunk's first
                            # scores pair is queued, so the exp stream never
                            # stalls at a chunk boundary
                            chunk_tail()
                            chunk_tail = None
                        elif j < jmax - 1:
                            # output-projection steps are held back for the
                            # LAST chunk: its exp stream is the longest (most
                            # k-tiles), which is exactly where the PE has
                            # slack to hide them. No pop at the section-last
                            # slot of long sections — a fill group there
                            # delays the NEXT section's first scores (and its
                            # exp); chunk 0's sections are too short to spare
                            # the slot.
                            pop_fill((qc, pr), allow_op=(qc >= QC - 1))
                    else:
                        last_e = eAB
                if stage < 3:
                    if pr == 0 and qc == 0:
                        nc.sync.dma_start(out_d[0:P, 0:SC], last_e[:, 0, :])
                    continue

                def chunk_tail(pend=pend, pr=pr, qc=qc, zA=zA, zB=zB,
                               emit_pv=emit_pv):
                    emit_pv(*pend)
                    normalize(pr, qc, zA, zB)
                    return pr, qc, zA, zB

            if stage >= 5:
                op_queue.extend(
                    (t, dh2)
                    for t in range(qc * TPC, (qc + 1) * TPC)
                    for dh2 in range(DH2)
                )
        def filler():
            # scratch matmul on long-finished data: the drain's PE gaps are
            # long enough for the HAM clock-gate to re-throttle (measured
            # 583-619ns N=512 matmuls there vs 376-379 in the flash); these
            # keep it at full clock so the real out-proj matmuls stay fast
            pfil = psz.tile([P, SC], f32, tag="z", name="pfil")
            nc.tensor.matmul(
                pfil[:], mm(ZTt[:, 0, 0:P]), mm(ZTt[:, 0, 0:SC]),
                start=True, stop=True,
            )

        if chunk_tail is not None:
            # final drain, pipelined: slice the last chunk's normalize per
            # q-tile and interleave that tile's out-proj steps, so the PE
            # overlaps the DVE multiplies instead of waiting for the full
            # chunk-width normalize
            pend, pr, qc, zA, zB = (chunk_tail.__defaults__[:5])
            chunk_tail.__defaults__[5](*pend)
            rb = rpool.tile([64, 2, SC], f32, tag="rb")
            ls = rpool.tile([64, 2, SC], f32, tag="ls")
            for ti in range(TPC):
                # whole DVE chain sliced per q-tile: each tile's out-proj
                # matmuls overlap the next tile's copies/reciprocal
                cl = slice(ti * P, (ti + 1) * P)
                qsl = slice(qc * SC + ti * P, qc * SC + (ti + 1) * P)
                nc.vector.tensor_copy(ls[:, 0, cl], zA[Dh : 2 * Dh, cl])
                nc.vector.tensor_copy(ls[:, 1, cl], zB[Dh : 2 * Dh, cl])
                nc.vector.reciprocal_approx_fast(rb[:, :, cl], ls[:, :, cl])
                nc.vector.tensor_mul(ZTt[0:64, pr, qsl], zA[0:Dh, cl], rb[:, 0, cl])
                nc.vector.tensor_mul(ZTt[64:128, pr, qsl], zB[0:Dh, cl], rb[:, 1, cl])
                # the drain's eviction copies go on the (now idle) Scalar
                # engine, keeping the Vector queue clear for the normalize
                # slices that gate them
                filler()
                for k in range(DH2):
                    if op_queue:
                        op_step(*op_queue.popleft(), evict_act=True)()
                filler()
        while op_queue or fill_q:
            pop_fill((QC, 0))
        assert not ot_stage, "unflushed output staging tiles"
        if stage == 4:
            nc.sync.dma_start(out_d[0:P, 0:Dm], ZTt[:, 0, 0:Dm])


def pack_inputs(x_b, W_Q, W_K, W_V, W_O, b_Q, b_K, hds, mm_mode):
    """Host-side packing of one core's shard into the kernel's layouts."""
    npdt = _np_sb(mm_mode)
    Dm, Dh = W_Q.shape[1], W_Q.shape[2]
    S = x_b.shape[0]
    NH = len(hds)
    NPAIR = NH // 2
    KT = Dm // P

    xT = np.ascontiguousarray(
        x_b.T.reshape(KT, P, S).transpose(1, 0, 2)
    ).astype(npdt)

    def pack_w_in(W):  # [H, Dm, Dh] -> [P, KT, NPAIR, 2*Dh]
        W4 = np.asarray(W)[hds]  # [NH, Dm, Dh]
        t = W4.reshape(NPAIR, 2, KT, P, Dh).transpose(3, 2, 0, 1, 4)
        return t.reshape(P, KT, NPAIR, 2 * Dh)

    wqk = np.ascontiguousarray(
        np.stack([pack_w_in(W_Q), pack_w_in(W_K)], axis=2)  # [P, KT, 2, NPAIR, 128]
    ).astype(npdt)

    WV4 = np.asarray(W_V)[hds]  # [NH, Dm, Dh]
    wv = np.ascontiguousarray(
        WV4.reshape(NH, KT, P, Dh).transpose(2, 1, 0, 3).reshape(P, KT, NH * Dh)
    ).astype(npdt)

    WO4 = np.asarray(W_O)[hds]  # [NH, Dh, Dm]
    wo = np.ascontiguousarray(
        WO4.reshape(NPAIR, 2, Dh, Dm).transpose(1, 2, 0, 3).reshape(P, NPAIR, Dm)
    ).astype(npdt)

    def pack_b(b):  # [H, Dh] -> [P, NPAIR]
        b4 = np.asarray(b)[hds]
        return b4.reshape(NPAIR, 2, Dh).transpose(1, 2, 0).reshape(P, NPAIR)

    bqk = np.ascontiguousarray(
        np.stack([pack_b(b_Q), pack_b(b_K)], axis=1)  # [P, 2, NPAIR]
    ).astype(np.float32)

    return {"xT": xT, "wqk": wqk, "wv": wv, "wo": wo, "bqk": bqk}


def kernel(x, W_Q, W_K, W_V, W_O, b_Q, b_K, b_V, b_O, _trace=False):
    from concourse.bass_utils import run_bass_kernel_spmd

    x = np.asarray(x, np.float32)
    B, S, Dm = x.shape
    H, _, Dh = W_Q.shape
    NCORES = 8
    GB = NCORES // B        # head groups per batch element
    NH = H // GB            # heads per core

    nc = build_nc(S, Dm, NH, Dh, MM_MODE)

    in_maps = []
    for c in range(NCORES):
        b, g = c // GB, c % GB
        hds = list(range(g * NH, (g + 1) * NH))
        in_maps.append(
            pack_inputs(x[b], W_Q, W_K, W_V, W_O, b_Q, b_K, hds, MM_MODE)
        )

    try:
        res = run_bass_kernel_spmd(
            nc, in_maps, core_ids=list(range(NCORES)), trace=_trace
        )
    except Exception:
        # transient device hiccups (e.g. a wedged core from a previous run)
        # usually clear on retry
        res = run_bass_kernel_spmd(
            nc, in_maps, core_ids=list(range(NCORES)), trace=_trace
        )

    out = np.zeros((B, S, Dm), np.float32)
    for c in range(NCORES):
        out[c // GB] += np.asarray(res.results[c]["out"], np.float32)

    # biases that commute out of the device kernel (softmax rows sum to 1)
    corr = np.asarray(b_O, np.float32) + np.einsum(
        "he,hed->d",
        np.asarray(b_V, np.float32),
        np.asarray(W_O, np.float32),
    )
    out += corr[None, None, :]

    if _trace:
        kernel.last_results = res
    return out
